# revision 1
# baseline (speedup 1.0000x reference)
"""Locally-connected graph-conv kernel for Trainium2 (Bass/Tile).

Computes out[b,t,m] = sum_n x[b,t,n] * (S*W)[n,m] + bias[m] for
x [64, 2048, 208], W/S [208, 208], bias [208].

The ring-graph support S is a +-4 band (mod 208), so each half of the
output nodes only needs a 112-row slice of the contraction dim:
  block 0 (m 0..103):   n in {204..207} ++ {0..107}
  block 1 (m 104..207): n in {100..207} ++ {0..3}
Each output block is then a SINGLE [112,104] x [112,512] fp32 matmul with
the masked-weight block stationary in the PE array and x^T streaming as
the moving operand in 512-column blocks (long streams hide the fp32
LDWEIGHTS). The bias is fused into the PSUM->SBUF eviction as a
per-partition tensor_scalar add on VectorE.

Data-parallel over 8 NeuronCores: each core gets 16384 rows of the
flattened x, host-pre-assembled into a [224, 16384] tensor (two 112-row
halo blocks). DMA partition counts are multiples of 16 (the fast HWDGE
path: ~250 GB/s/instr vs ~27 otherwise); stores are [112]-row DMAs into
a [224, SHARD] output (8 pad rows per block, dropped at host gather).
x loads issue on the Sync HWDGE ring, stores on the Scalar ring, one-time
weight/bias setup on the GpSimd SWDGE queue so it never delays them.
The host transposes y^T back at gather.
"""

import numpy as np
from contextlib import ExitStack

import concourse.bacc as bacc
import concourse.mybir as mybir
import concourse.tile as tile
from concourse.bass_utils import run_bass_kernel_spmd

N = 208                      # nodes
HALF = 104                   # output nodes per block
K = 4                        # band half-width of S
NH = 2 * K + HALF            # 112 contraction rows per block (halo incl.)
NP = 112                     # padded store rows (multiple of 16)
N_CORES = 8
B, T = 64, 2048
ROWS_TOTAL = B * T           # 131072
SHARD = ROWS_TOTAL // N_CORES    # 16384 rows per core
TB = 512                     # moving-block columns per matmul (fp32 PSUM max)
TB2 = 2 * TB                 # eviction group (2 PSUM banks)
TOUT = 2048                  # t-columns per DMA chunk (~0.9 MB loads)
N_CHUNKS = SHARD // TOUT     # 8
SUB = TOUT // TB2            # 2 psum groups per chunk

FP32 = mybir.dt.float32

# halo row order (indices into the [208] node dim) for each block
ROWS0 = list(range(N - K, N)) + list(range(0, HALF + K))          # 112
ROWS1 = list(range(HALF - K, N)) + list(range(0, K))              # 112

_CACHE = {}
LAST_RESULTS = None          # BassKernelResults of the most recent run


def _kernel_body(tc):
    nc = tc.nc
    # rows 0:112 block0 halo, 112:224 block1 halo
    x_d = nc.dram_tensor("xh", [2 * NH, SHARD], FP32, kind="ExternalInput").ap()
    w_d = nc.dram_tensor("w", [N, N], FP32, kind="ExternalInput").ap()
    s_d = nc.dram_tensor("s", [N, N], FP32, kind="ExternalInput").ap()
    b_d = nc.dram_tensor("bias", [1, N], FP32, kind="ExternalInput").ap()
    o_d = nc.dram_tensor("outt", [2 * NP, SHARD], FP32, kind="ExternalOutput").ap()

    with ExitStack() as ctx:
        const = ctx.enter_context(tc.tile_pool(name="const", bufs=1))

        # One-time setup: w/s pieces on the Scalar HWDGE ring (fast issue,
        # idle at startup), bias on GpSimd. Stationary blocks wh0/wh1
        # [112, 104]: masked weight rows in halo order. Bias [104, 1].
        w0 = const.tile([NH, HALF], FP32, tag="w0")
        s0 = const.tile([NH, HALF], FP32, tag="s0")
        nc.scalar.dma_start(w0[0:K, :], w_d[N - K : N, 0:HALF])
        nc.scalar.dma_start(w0[K:NH, :], w_d[0 : HALF + K, 0:HALF])
        nc.scalar.dma_start(s0[0:K, :], s_d[N - K : N, 0:HALF])
        nc.scalar.dma_start(s0[K:NH, :], s_d[0 : HALF + K, 0:HALF])
        wh0 = const.tile([NH, HALF], FP32, tag="wh0")
        nc.vector.tensor_mul(wh0, w0, s0)
        w1 = const.tile([NH, HALF], FP32, tag="w1")
        s1 = const.tile([NH, HALF], FP32, tag="s1")
        nc.scalar.dma_start(w1[0 : HALF + K, :], w_d[HALF - K : N, HALF:N])
        nc.scalar.dma_start(w1[HALF + K : NH, :], w_d[0:K, HALF:N])
        nc.scalar.dma_start(s1[0 : HALF + K, :], s_d[HALF - K : N, HALF:N])
        nc.scalar.dma_start(s1[HALF + K : NH, :], s_d[0:K, HALF:N])
        wh1 = const.tile([NH, HALF], FP32, tag="wh1")
        nc.vector.tensor_mul(wh1, w1, s1)
        bA = const.tile([HALF, 1], FP32, tag="bA")
        bB = const.tile([HALF, 1], FP32, tag="bB")
        b_col = b_d.rearrange("o n -> n o")
        nc.gpsimd.dma_start(bA, b_col[0:HALF, :])
        nc.gpsimd.dma_start(bB, b_col[HALF:N, :])

        x0p = ctx.enter_context(tc.tile_pool(name="x0p", bufs=6))
        x1p = ctx.enter_context(tc.tile_pool(name="x1p", bufs=6))
        o0p = ctx.enter_context(tc.tile_pool(name="o0p", bufs=4))
        o1p = ctx.enter_context(tc.tile_pool(name="o1p", bufs=4))
        ps0p = ctx.enter_context(tc.tile_pool(name="ps0p", bufs=2, space="PSUM"))
        ps1p = ctx.enter_context(tc.tile_pool(name="ps1p", bufs=2, space="PSUM"))

        for c in range(N_CHUNKS):
            tsl = slice(c * TOUT, (c + 1) * TOUT)
            xh0 = x0p.tile([NH, TOUT], FP32, tag="xh0")
            xh1 = x1p.tile([NH, TOUT], FP32, tag="xh1")
            if c == 0:
                # split the critical-path first loads for 2x DMA concurrency
                nc.sync.dma_start(xh0[0:64, :], x_d[0:64, tsl])
                nc.sync.dma_start(xh0[64:NH, :], x_d[64:NH, tsl])
                nc.sync.dma_start(xh1[0:64, :], x_d[NH : NH + 64, tsl])
                nc.sync.dma_start(xh1[64:NH, :], x_d[NH + 64 : 2 * NH, tsl])
            else:
                nc.sync.dma_start(xh0, x_d[0:NH, tsl])
                nc.sync.dma_start(xh1, x_d[NH : 2 * NH, tsl])

            o0_t = o0p.tile([NP, TOUT], FP32, tag="o0")
            o1_t = o1p.tile([NP, TOUT], FP32, tag="o1")
            for s in range(SUB):
                g = slice(s * TB2, (s + 1) * TB2)
                ga = slice(s * TB2, s * TB2 + TB)
                gb = slice(s * TB2 + TB, (s + 1) * TB2)
                # [104, 1024] PSUM tiles (2 banks); each matmul fills one bank
                ps0 = ps0p.tile([HALF, TB2], FP32, tag="ps0")
                nc.tensor.matmul(ps0[:, 0:TB], wh0, xh0[:, ga], start=True, stop=True)
                nc.tensor.matmul(ps0[:, TB:TB2], wh0, xh0[:, gb], start=True, stop=True)
                ps1 = ps1p.tile([HALF, TB2], FP32, tag="ps1")
                nc.tensor.matmul(ps1[:, 0:TB], wh1, xh1[:, ga], start=True, stop=True)
                nc.tensor.matmul(ps1[:, TB:TB2], wh1, xh1[:, gb], start=True, stop=True)
                # eviction + per-partition bias on VectorE
                nc.vector.tensor_scalar_add(o0_t[0:HALF, g], ps0, bA)
                nc.vector.tensor_scalar_add(o1_t[0:HALF, g], ps1, bB)
            # per-chunk stores (112 rows, 8 pad) on the Scalar HWDGE ring;
            # the last chunk's second-block store rides the by-then-idle Sync
            # ring so the two tail stores run in parallel
            nc.scalar.dma_start(o_d[0:NP, tsl], o0_t)
            if c == N_CHUNKS - 1:
                nc.sync.dma_start(o_d[NP : 2 * NP, tsl], o1_t)
            else:
                nc.scalar.dma_start(o_d[NP : 2 * NP, tsl], o1_t)


def _build():
    nc = bacc.Bacc(
        "TRN2",
        target_bir_lowering=False,
        debug=False,
        num_devices=N_CORES,
    )
    with tile.TileContext(nc) as tc:
        _kernel_body(tc)
    nc.compile()
    return nc


def kernel(x, W, b, S):
    global LAST_RESULTS
    nc = _CACHE.get("nc")
    if nc is None:
        nc = _build()
        _CACHE["nc"] = nc

    xf = np.asarray(x, np.float32).reshape(ROWS_TOTAL, N)
    Wf = np.ascontiguousarray(np.asarray(W, np.float32))
    Sf = np.ascontiguousarray(np.asarray(S, np.float32))
    bf = np.ascontiguousarray(np.asarray(b, np.float32).reshape(1, N))

    in_maps = []
    for i in range(N_CORES):
        xt = xf[i * SHARD : (i + 1) * SHARD].T          # [208, SHARD] view
        xh = np.empty((2 * NH, SHARD), np.float32)
        xh[0:NH] = xt[ROWS0]
        xh[NH : 2 * NH] = xt[ROWS1]
        in_maps.append({"xh": xh, "w": Wf, "s": Sf, "bias": bf})
    res = run_bass_kernel_spmd(nc, in_maps, core_ids=list(range(N_CORES)))
    LAST_RESULTS = res
    out = np.empty((ROWS_TOTAL, N), np.float32)
    for i, r in enumerate(res.results):
        yt = r["outt"]                                  # [224, SHARD]
        out[i * SHARD : (i + 1) * SHARD, 0:HALF] = yt[0:HALF].T
        out[i * SHARD : (i + 1) * SHARD, HALF:N] = yt[NP : NP + HALF].T
    return out.reshape(B, T, N)



# revision 2
# speedup vs baseline: 1.6287x; 1.6287x over previous
"""Locally-connected graph-conv kernel for Trainium2 (Bass/Tile), bf16.

Computes out[b,t,m] = sum_n x[b,t,n] * (S*W)[n,m] + bias[m] for
x [64, 2048, 208], W/S [208, 208], bias [208].

The ring-graph support S is a +-4 band (mod 208), so each half of the
output nodes only needs a 112-row slice of the contraction dim:
  block 0 (m 0..103):   n in {204..207} ++ {0..107}
  block 1 (m 104..207): n in {100..207} ++ {0..3}

v2 (bf16): the fp32 baseline was tensor-bound (fp32 matmul streams at
~1/4 rate; 94.6us of MATMUL in a 109us kernel) and DMA-bound (29.5MB
at ~360GB/s aggregate over 16 DMA engines). Tolerance is 2e-2 and bf16
end-to-end measures 4.5e-3 max rel err, so: the host pre-casts x and
the masked weight to bf16 (halves HBM load traffic AND quadruples PE
column rate), the kernel stores bf16 (halves store traffic), and the
host upcasts on gather. PSUM eviction+bias is split across VectorE
(block 0) and ScalarE/ACT (block 1) so neither engine is the new
bottleneck. Expected ~41us of DMA at the same queue rates.

Data-parallel over 8 NeuronCores: each core gets 16384 rows of the
flattened x, host-pre-assembled into a bf16 [224, 16384] tensor (two
112-row halo blocks; partition counts multiple of 16 keep the fast
HWDGE striping). Loads ride the Sync HWDGE ring, stores the Scalar
ring; the first chunk's loads and last chunk's stores are split
column-wise across both rings to shorten pipeline head/tail.
"""

import numpy as np
from contextlib import ExitStack

import concourse.bacc as bacc
import concourse.mybir as mybir
import concourse.tile as tile
from concourse.bass_utils import run_bass_kernel_spmd

N = 208                      # nodes
HALF = 104                   # output nodes per block
K = 4                        # band half-width of S
NH = 2 * K + HALF            # 112 contraction rows per block (halo incl.)
NP = 112                     # padded store rows (multiple of 16)
N_CORES = 8
B, T = 64, 2048
ROWS_TOTAL = B * T           # 131072
SHARD = ROWS_TOTAL // N_CORES    # 16384 rows per core
TB = 512                     # moving-block columns per matmul (fp32 PSUM bank)
TG = 2 * TB                  # psum tile / eviction group (2 banks)
TOUT = 4096                  # t-columns per DMA chunk (0.92 MB bf16 loads)
N_CHUNKS = SHARD // TOUT     # 4
SUB = TOUT // TG             # 4 psum groups per chunk per block

FP32 = mybir.dt.float32
BF16 = mybir.dt.bfloat16
BF16_NP = mybir.dt.np(BF16)

# halo row order (indices into the [208] node dim) for each block
ROWS0 = list(range(N - K, N)) + list(range(0, HALF + K))          # 112
ROWS1 = list(range(HALF - K, N)) + list(range(0, K))              # 112

_CACHE = {}
LAST_RESULTS = None          # BassKernelResults of the most recent run


def _kernel_body(tc):
    nc = tc.nc
    # rows 0:112 block0 halo, 112:224 block1 halo
    x_d = nc.dram_tensor("xh", [2 * NH, SHARD], BF16, kind="ExternalInput").ap()
    w_d = nc.dram_tensor("wh", [NH, N], BF16, kind="ExternalInput").ap()
    b_d = nc.dram_tensor("bias", [1, N], FP32, kind="ExternalInput").ap()
    o_d = nc.dram_tensor("outt", [2 * NP, SHARD], BF16, kind="ExternalOutput").ap()

    with ExitStack() as ctx:
        const = ctx.enter_context(tc.tile_pool(name="const", bufs=1))

        # One-time setup on the GpSimd SWDGE queue so the HWDGE rings are
        # free for the first x loads. wh [112, 208] bf16: cols 0:104 are
        # the block-0 masked weights in halo-row order, 104:208 block 1.
        wh = const.tile([NH, N], BF16, tag="wh")
        nc.gpsimd.dma_start(wh, w_d)
        bA = const.tile([HALF, 1], FP32, tag="bA")
        bB = const.tile([HALF, 1], FP32, tag="bB")
        b_col = b_d.rearrange("o n -> n o")
        nc.gpsimd.dma_start(bA, b_col[0:HALF, :])
        nc.gpsimd.dma_start(bB, b_col[HALF:N, :])
        wh0 = wh[:, 0:HALF]
        wh1 = wh[:, HALF:N]

        x0p = ctx.enter_context(tc.tile_pool(name="x0p", bufs=3))
        x1p = ctx.enter_context(tc.tile_pool(name="x1p", bufs=3))
        o0p = ctx.enter_context(tc.tile_pool(name="o0p", bufs=2))
        o1p = ctx.enter_context(tc.tile_pool(name="o1p", bufs=2))
        ps0p = ctx.enter_context(tc.tile_pool(name="ps0p", bufs=2, space="PSUM"))
        ps1p = ctx.enter_context(tc.tile_pool(name="ps1p", bufs=2, space="PSUM"))

        for c in range(N_CHUNKS):
            tsl = slice(c * TOUT, (c + 1) * TOUT)
            xh0 = x0p.tile([NH, TOUT], BF16, tag="xh0")
            xh1 = x1p.tile([NH, TOUT], BF16, tag="xh1")
            if c == 0:
                # head: split the critical first loads column-wise across
                # both HWDGE rings so the first matmuls start ~3x earlier
                h = TOUT // 2
                nc.sync.dma_start(xh0[:, 0:h], x_d[0:NH, 0:h])
                nc.scalar.dma_start(xh1[:, 0:h], x_d[NH : 2 * NH, 0:h])
                nc.sync.dma_start(xh0[:, h:TOUT], x_d[0:NH, h:TOUT])
                nc.scalar.dma_start(xh1[:, h:TOUT], x_d[NH : 2 * NH, h:TOUT])
            else:
                nc.sync.dma_start(xh0, x_d[0:NH, tsl])
                nc.sync.dma_start(xh1, x_d[NH : 2 * NH, tsl])

            o0_t = o0p.tile([NP, TOUT], BF16, tag="o0")
            o1_t = o1p.tile([NP, TOUT], BF16, tag="o1")
            # all block-0 matmuls back-to-back (stationary wh0 stays in the
            # PE array), then all block-1; evictions chase on Vector/Scalar
            ps0s, ps1s = [], []
            for s in range(SUB):
                ga = slice(s * TG, s * TG + TB)
                gb = slice(s * TG + TB, (s + 1) * TG)
                ps0 = ps0p.tile([HALF, TG], FP32, tag="ps0")
                nc.tensor.matmul(ps0[:, 0:TB], wh0, xh0[:, ga], start=True, stop=True)
                nc.tensor.matmul(ps0[:, TB:TG], wh0, xh0[:, gb], start=True, stop=True)
                ps0s.append(ps0)
                nc.vector.tensor_scalar_add(
                    o0_t[0:HALF, s * TG : (s + 1) * TG], ps0, bA
                )
            for s in range(SUB):
                ga = slice(s * TG, s * TG + TB)
                gb = slice(s * TG + TB, (s + 1) * TG)
                ps1 = ps1p.tile([HALF, TG], FP32, tag="ps1")
                nc.tensor.matmul(ps1[:, 0:TB], wh1, xh1[:, ga], start=True, stop=True)
                nc.tensor.matmul(ps1[:, TB:TG], wh1, xh1[:, gb], start=True, stop=True)
                ps1s.append(ps1)
                nc.scalar.add(o1_t[0:HALF, s * TG : (s + 1) * TG], ps1, bB)

            if c == N_CHUNKS - 1:
                # tail: last stores split column-wise across both rings
                h = TOUT // 2
                nc.scalar.dma_start(o_d[0:NP, c * TOUT : c * TOUT + h], o0_t[:, 0:h])
                nc.sync.dma_start(
                    o_d[0:NP, c * TOUT + h : (c + 1) * TOUT], o0_t[:, h:TOUT]
                )
                nc.scalar.dma_start(
                    o_d[NP : 2 * NP, c * TOUT : c * TOUT + h], o1_t[:, 0:h]
                )
                nc.sync.dma_start(
                    o_d[NP : 2 * NP, c * TOUT + h : (c + 1) * TOUT], o1_t[:, h:TOUT]
                )
            else:
                nc.scalar.dma_start(o_d[0:NP, tsl], o0_t)
                nc.scalar.dma_start(o_d[NP : 2 * NP, tsl], o1_t)


def _build():
    nc = bacc.Bacc(
        "TRN2",
        target_bir_lowering=False,
        debug=False,
        num_devices=N_CORES,
    )
    with tile.TileContext(nc) as tc:
        _kernel_body(tc)
    nc.compile()
    return nc


def kernel(x, W, b, S):
    global LAST_RESULTS
    nc = _CACHE.get("nc")
    if nc is None:
        nc = _build()
        _CACHE["nc"] = nc

    xf = np.asarray(x, np.float32).reshape(ROWS_TOTAL, N)
    WS = np.asarray(S, np.float32) * np.asarray(W, np.float32)
    wh = np.empty((NH, N), BF16_NP)
    wh[:, 0:HALF] = WS[ROWS0][:, 0:HALF].astype(BF16_NP)
    wh[:, HALF:N] = WS[ROWS1][:, HALF:N].astype(BF16_NP)
    bf = np.ascontiguousarray(np.asarray(b, np.float32).reshape(1, N))

    xt = np.ascontiguousarray(xf.T).astype(BF16_NP)      # [208, 131072] bf16
    in_maps = []
    for i in range(N_CORES):
        sl = slice(i * SHARD, (i + 1) * SHARD)
        xh = np.empty((2 * NH, SHARD), BF16_NP)
        xh[0:NH] = xt[ROWS0, sl]
        xh[NH : 2 * NH] = xt[ROWS1, sl]
        in_maps.append({"xh": xh, "wh": wh, "bias": bf})
    res = run_bass_kernel_spmd(nc, in_maps, core_ids=list(range(N_CORES)))
    LAST_RESULTS = res
    out = np.empty((ROWS_TOTAL, N), np.float32)
    for i, r in enumerate(res.results):
        yt = r["outt"]                                   # [224, SHARD] bf16
        out[i * SHARD : (i + 1) * SHARD, 0:HALF] = yt[0:HALF].T.astype(np.float32)
        out[i * SHARD : (i + 1) * SHARD, HALF:N] = yt[NP : NP + HALF].T.astype(
            np.float32
        )
    return out.reshape(B, T, N)
